# revision 8
# baseline (speedup 1.0000x reference)
"""KMeans cross-attention kernel for Trainium2 (8 NeuronCores).

Math (per batch b):
  logits[n, m] = sum_d q[n,d] * k[m,d]
  idx[m] = argmax_n logits[n, m]
  out[n, :] = (sum_{m: idx[m]=n} v[m, :]) / (max(count[n], 1) + 1e-6)

Sharding: keys (M) are split across the 8 cores; q is replicated. Each core
computes, for its key shard, per-key argmax over the 1024 queries as an
exact one-hot indicator (exp(2^27*(x - max)) == 1 iff x == max, else 0,
because 2^27 * (x - max) <= -2^27*ulp(max) <= -512 for any non-max x), then
scatter-adds v and counts via matmuls against that one-hot, all-reduces the
(out_T, counts) partials across cores, and divides.

Host-side prep only re-lays-out the inputs (k/q pre-transposed so that D is
the partition dim); all FLOPs + reduction + division happen on device.
"""

import sys

for _p in ("/opt/trn_rl_repo",):
    if _p not in sys.path:
        sys.path.insert(0, _p)

import numpy as np

import concourse.bass as bass
import concourse.bacc as bacc
import concourse.mybir as mybir
import concourse.tile as tile
from concourse.bass_utils import run_bass_kernel_spmd

B, N, M, D = 2, 1024, 65536, 128
NCORES = 8
MS = M // NCORES          # keys per core per batch (8192)
CHUNK = 2048              # keys per DMA chunk
NT = CHUNK // 128         # key-tiles per chunk
NCH = MS // CHUNK         # chunks per batch
F32 = mybir.dt.float32
SCALE = float(2 ** 27)    # power of two -> exact scaling in fp32
EPS = 1e-6

_CACHE = {}


def build_kernel():
    nc = bacc.Bacc("TRN2", target_bir_lowering=False, debug=False,
                   num_devices=NCORES)
    qT = nc.dram_tensor("qT", [B, D, N], F32, kind="ExternalInput")
    kT = nc.dram_tensor("kT", [B, D, MS], F32, kind="ExternalInput")
    vS = nc.dram_tensor("vS", [B, MS, D], F32, kind="ExternalInput")
    ident = nc.dram_tensor("ident", [D, D], F32, kind="ExternalInput")
    out = nc.dram_tensor("out", [B, N, D], F32, kind="ExternalOutput")

    AF = mybir.ActivationFunctionType
    AX = mybir.AxisListType

    with tile.TileContext(nc) as tc:
        with (
            tc.tile_pool(name="const", bufs=1) as constp,
            tc.tile_pool(name="kv", bufs=3) as kvp,
            tc.tile_pool(name="oh", bufs=4) as ohp,
            tc.tile_pool(name="small", bufs=8) as smallp,
            tc.tile_pool(name="post", bufs=2) as postp,
            tc.tile_pool(name="dram", bufs=1, space="DRAM") as dramp,
        ):
            # ---- constants ----
            qt_sb = []
            for b in range(B):
                t = constp.tile([D, N], F32, tag=f"qt{b}")
                nc.sync.dma_start(t[:], qT[b])
                qt_sb.append(t)
            id_sb = constp.tile([D, D], F32, tag="ident")
            nc.sync.dma_start(id_sb[:], ident[:])
            ones_sb = constp.tile([128, 1], F32, tag="ones")
            nc.vector.memset(ones_sb[:], 1.0)

            cc_in = dramp.tile([B, D + 1, N], F32)
            cc_out = dramp.tile([B, D + 1, N], F32)

            # ---- main loop: per batch, stream key tiles ----
            with (
                tc.tile_pool(name="lg", bufs=2, space="PSUM") as lgp,
                tc.tile_pool(name="acc", bufs=1, space="PSUM") as accp,
            ):
                for b in range(B):
                    # acc cols 0:1024 = out_T accumulate (banks 0-1),
                    # [0:1, 1024:2048] = counts accumulate (banks 2-3)
                    acc = accp.tile([128, 2048], F32, tag="acc")
                    ntile = NCH * NT
                    for c in range(NCH):
                        kt_ch = kvp.tile([D, CHUNK], F32, tag="kt")
                        v_ch = kvp.tile([128, NT, 128], F32, tag="v")
                        nc.sync.dma_start(
                            kt_ch[:], kT[b, :, c * CHUNK:(c + 1) * CHUNK])
                        nc.sync.dma_start(
                            v_ch[:],
                            vS[b, c * CHUNK:(c + 1) * CHUNK, :]
                            .rearrange("(t p) d -> p t d", p=128))
                        for t in range(NT):
                            i = c * NT + t
                            ksl = kt_ch[:, t * 128:(t + 1) * 128]
                            vsl = v_ch[:, t, :]
                            lg = lgp.tile([128, N], F32, tag="lg")
                            # logits^T tile: [128 keys, 1024 queries]
                            nc.tensor.matmul(lg[:, 0:512], ksl,
                                             qt_sb[b][:, 0:512],
                                             start=True, stop=True)
                            nc.tensor.matmul(lg[:, 512:1024], ksl,
                                             qt_sb[b][:, 512:1024],
                                             start=True, stop=True)
                            mx = smallp.tile([128, 1], F32, tag="mx")
                            nc.vector.reduce_max(mx[:], lg[:], axis=AX.X)
                            nb = smallp.tile([128, 1], F32, tag="nb")
                            nc.vector.tensor_scalar_mul(nb[:], mx[:], -SCALE)
                            oh = ohp.tile([128, N], F32, tag="oh")
                            # exact one-hot: exp(2^27*x - 2^27*max)
                            nc.scalar.activation(oh[:], lg[:], AF.Exp,
                                                 bias=nb[:], scale=SCALE)
                            st = (i == 0)
                            sp = (i == ntile - 1)
                            # out_T += v^T @ onehot   [128 D, 1024 n]
                            nc.tensor.matmul(acc[:, 0:512], vsl, oh[:, 0:512],
                                             start=st, stop=sp)
                            nc.tensor.matmul(acc[:, 512:1024], vsl,
                                             oh[:, 512:1024],
                                             start=st, stop=sp)
                            # counts += ones^T @ onehot  [1, 1024]
                            nc.tensor.matmul(acc[0:1, 1024:1536], ones_sb[:],
                                             oh[:, 0:512], start=st, stop=sp)
                            nc.tensor.matmul(acc[0:1, 1536:2048], ones_sb[:],
                                             oh[:, 512:1024],
                                             start=st, stop=sp)
                    stg = postp.tile([D, N], F32, tag="stg")
                    nc.scalar.activation(stg[:], acc[:, 0:1024], AF.Copy,
                                         bias=0.0, scale=1.0)
                    stc = postp.tile([1, N], F32, tag="stc")
                    nc.vector.tensor_copy(stc[:], acc[0:1, 1024:2048])
                    nc.sync.dma_start(cc_in[b, 0:D, :], stg[:])
                    nc.sync.dma_start(cc_in[b, D, :], stc[:])

            # ---- all-reduce partials across the 8 cores ----
            nc.gpsimd.collective_compute(
                "AllReduce",
                mybir.AluOpType.add,
                replica_groups=[list(range(NCORES))],
                ins=[cc_in.opt()],
                outs=[cc_out.opt()],
            )

            # ---- divide by counts and transpose to [N, D] ----
            with tc.tile_pool(name="tp", bufs=2, space="PSUM") as tpp:
                for b in range(B):
                    rT = postp.tile([D, N], F32, tag="rT")
                    nc.sync.dma_start(rT[:], cc_out[b, 0:D, :])
                    ct = postp.tile([128, 8], F32, tag="ct")
                    # counts[j*128+p] -> ct[p, j] (transposing dram read)
                    nc.sync.dma_start(
                        ct[:], cc_out[b, D, :].rearrange("(j p) -> p j", p=128))
                    dn = postp.tile([128, 8], F32, tag="dn")
                    nc.vector.tensor_scalar_max(dn[:], ct[:], 1.0)
                    nc.vector.tensor_scalar_add(dn[:], dn[:], EPS)
                    rc = postp.tile([128, 8], F32, tag="rc")
                    nc.vector.reciprocal(rc[:], dn[:])
                    for j in range(8):
                        tp = tpp.tile([128, 128], F32, tag="tp")
                        nc.tensor.transpose(
                            tp[:], rT[:, j * 128:(j + 1) * 128], id_sb[:])
                        fo = postp.tile([128, 128], F32, tag="fo")
                        nc.scalar.activation(fo[:], tp[:], AF.Copy,
                                             bias=0.0, scale=rc[:, j:j + 1])
                        nc.sync.dma_start(
                            out[b, j * 128:(j + 1) * 128, :], fo[:])
    nc.compile()
    return nc


def _get_nc():
    if "nc" not in _CACHE:
        _CACHE["nc"] = build_kernel()
    return _CACHE["nc"]


def _in_maps(q, k, v):
    q = np.ascontiguousarray(np.asarray(q, dtype=np.float32))
    k = np.ascontiguousarray(np.asarray(k, dtype=np.float32))
    v = np.ascontiguousarray(np.asarray(v, dtype=np.float32))
    qT = np.ascontiguousarray(q.transpose(0, 2, 1))
    ident = np.eye(D, dtype=np.float32)
    maps = []
    for c in range(NCORES):
        sl = slice(c * MS, (c + 1) * MS)
        maps.append({
            "qT": qT,
            "kT": np.ascontiguousarray(k[:, sl, :].transpose(0, 2, 1)),
            "vS": np.ascontiguousarray(v[:, sl, :]),
            "ident": ident,
        })
    return maps


def kernel(q, k, v):
    nc = _get_nc()
    res = run_bass_kernel_spmd(nc, _in_maps(q, k, v),
                               core_ids=list(range(NCORES)))
    return np.asarray(res.results[0]["out"])


def kernel_profiled(q, k, v):
    """Returns (output, BassKernelResults) with trace enabled."""
    nc = _get_nc()
    res = run_bass_kernel_spmd(nc, _in_maps(q, k, v),
                               core_ids=list(range(NCORES)), trace=True)
    return np.asarray(res.results[0]["out"]), res
